# revision 1
# baseline (speedup 1.0000x reference)
"""GATv2 (3 layers, self-loops, segment softmax) on 8 Trainium2 NeuronCores.

Strategy (per spec sharding hint): nodes sharded contiguously across 8 cores;
edges routed to the core owning their dst; per core, edges sorted by dst and
grouped into 128-dst blocks x 128-edge chunks; per layer each core computes
xl/xr for its shard (PE), AllGathers the xl table, then runs the edge phase:
indirect-DMA gathers of xl[src]/xr[dst], score computation on DVE/ACT, and a
one-hot matmul (PE) that does the per-block segment reduction of both the
softmax numerator and denominator in one PSUM accumulation.

Self-contained: hardcodes problem shapes; no sibling imports.
"""
import numpy as np

P = 128          # partitions / block size / chunk size
SC = 4           # chunks per superchunk (batched gathers + elementwise)
NEG_SLOPE = 0.2


# ---------------------------------------------------------------- host prep

def prep_edges(src, dst, N, ncores):
    """Route edges to dst-owning cores, sort by dst, pack into block/chunk slots.

    Returns (Nshard, nblk, NSC, ids) where ids is int32
    [ncores, nblk, NSC, P, 3*SC]: cols [0:SC] global src id, [SC:2*SC] local
    dst id, [2*SC:3*SC] float32-bitcast block-relative dst (1e6 sentinel pad).
    """
    Nshard = ((N + ncores * P - 1) // (ncores * P)) * P
    nblk = Nshard // P
    core = dst // Nshard
    percore = []
    maxch = 1
    for c in range(ncores):
        m = core == c
        s = src[m]
        dl = (dst[m] - c * Nshard).astype(np.int64)
        o = np.argsort(dl, kind='stable')
        s, dl = s[o], dl[o]
        blk = dl // P
        counts = np.bincount(blk, minlength=nblk)
        maxch = max(maxch, int(np.max((counts + P - 1) // P)))
        percore.append((s, dl, blk, counts))
    NSC = (maxch + SC - 1) // SC
    CH = NSC * SC
    ids = np.zeros((ncores, nblk, CH, P, 2), np.int32)
    dst16 = np.zeros((ncores, nblk, CH * P), np.int16)   # block-local dst for dma_gather
    sentinel = np.float32(1e6).view(np.int32)
    ids[:, :, :, :, 1] = sentinel
    for c in range(ncores):
        s, dl, blk, counts = percore[c]
        starts = np.zeros(nblk, np.int64)
        starts[1:] = np.cumsum(counts)[:-1]
        pos = np.arange(len(s)) - starts[blk]        # rank within block
        ch = pos // P
        p = pos % P
        ids[c, blk, ch, p, 0] = s
        ids[c, blk, ch, p, 1] = (dl - blk * P).astype(np.float32).view(np.int32)
        dst16[c, blk, ch * P + p] = dl
    # ids: [nblk, CH, P, 2] -> [nblk, NSC, P, 2*SC] with cols grouped (src*SC, rel*SC)
    ids = ids.reshape(ncores, nblk, NSC, SC, P, 2)
    ids = ids.transpose(0, 1, 2, 4, 5, 3)            # [c, nblk, NSC, P, 2, SC]
    ids = ids.reshape(ncores, nblk, NSC, P, 2 * SC)
    # dst16: wrap for dma_gather: idx i read from [i % 16, i // 16], x8 partitions
    n = CH * P
    dst16 = dst16.reshape(ncores, nblk, n // 16, 16).transpose(0, 1, 3, 2)  # [c,b,16,n/16]
    dst16 = np.tile(dst16, (1, 1, 8, 1))             # [c, nblk, 128, n/16]
    return Nshard, nblk, NSC, np.ascontiguousarray(ids), np.ascontiguousarray(dst16)


# ---------------------------------------------------------------- bass build

def build_program(ncores, Nshard, nblk, NSC, dims_in, H, C, use_collective=True):
    import concourse.bass as bass
    import concourse.mybir as mybir
    from concourse import bacc
    from concourse.tile import TileContext

    D = H * C
    W = D + H
    L = len(dims_in)
    CH = NSC * SC
    Np = Nshard * ncores
    f32, i32 = mybir.dt.float32, mybir.dt.int32
    AF = mybir.ActivationFunctionType
    OP = mybir.AluOpType

    i16 = mybir.dt.int16
    from concourse import library_config

    nc = bacc.Bacc()
    x0 = nc.declare_dram_parameter("x0", [Nshard, dims_in[0]], f32, isOutput=False)
    ids = nc.declare_dram_parameter("ids", [nblk, NSC, P, 2 * SC], i32, isOutput=False)
    dst16 = nc.declare_dram_parameter("dst16", [nblk, P, CH * P // 16], i16, isOutput=False)
    wparams = []
    for l in range(L):
        din = dims_in[l]
        wparams.append((
            nc.declare_dram_parameter(f"Wl{l}", [din, D], f32, isOutput=False),
            nc.declare_dram_parameter(f"Wr{l}", [din, P], f32, isOutput=False),
            nc.declare_dram_parameter(f"attb{l}", [P, D], f32, isOutput=False),
            nc.declare_dram_parameter(f"biasb{l}", [P, D], f32, isOutput=False),
        ))
    ident_in = nc.declare_dram_parameter("ident", [P, P], f32, isOutput=False)
    iota_in = nc.declare_dram_parameter("iota", [P, P], f32, isOutput=False)
    y = nc.declare_dram_parameter("y", [Nshard, D], f32, isOutput=True)

    xl_sh = nc.dram_tensor("xl_sh", [Nshard, D], f32)
    xl_full = nc.dram_tensor("xl_full", [Np, D], f32, addr_space="Shared")
    xr_tab = nc.dram_tensor("xr_tab", [Nshard, P], f32)   # rows padded to 512B for dma_gather
    xmid = [nc.dram_tensor(f"xmid{i}", [Nshard, D], f32) for i in range(L - 1)]

    with TileContext(nc) as tc:
        with (
            tc.tile_pool(name="const", bufs=1) as cp,
            tc.tile_pool(name="ab", bufs=3) as abp,
            tc.tile_pool(name="abps", bufs=2, space="PSUM") as abps,
            tc.tile_pool(name="edge", bufs=3) as ep,
            tc.tile_pool(name="blkps", bufs=2, space="PSUM") as blkps,
            tc.tile_pool(name="fin", bufs=2) as fp,
        ):
            nc.gpsimd.load_library(library_config.mlp)
            ident = cp.tile([P, P], f32, tag="ident")
            nc.sync.dma_start(out=ident[:], in_=ident_in[:])
            iota = cp.tile([P, P], f32, tag="iota")
            nc.sync.dma_start(out=iota[:], in_=iota_in[:])
            wts = []
            for l in range(L):
                din = dims_in[l]
                wl = cp.tile([din, D], f32, tag=f"wl{l}")
                nc.sync.dma_start(out=wl[:], in_=wparams[l][0][:])
                wr = cp.tile([din, P], f32, tag=f"wr{l}")
                nc.sync.dma_start(out=wr[:], in_=wparams[l][1][:])
                attb = cp.tile([P, D], f32, tag=f"attb{l}")
                nc.sync.dma_start(out=attb[:], in_=wparams[l][2][:])
                biasb = cp.tile([P, D], f32, tag=f"biasb{l}")
                nc.sync.dma_start(out=biasb[:], in_=wparams[l][3][:])
                wts.append((wl, wr, attb, biasb))

            for l in range(L):
                din = dims_in[l]
                x_cur = x0 if l == 0 else xmid[l - 1]
                x_out = y if l == L - 1 else xmid[l]
                wl, wr, attb, biasb = wts[l]

                # ---- node transform: xl_sh = x @ Wl, xr_tab = x @ Wr
                for t in range(nblk):
                    xc = abp.tile([P, din], f32, tag="xc")
                    nc.sync.dma_start(out=xc[:], in_=x_cur[t * P:(t + 1) * P, :])
                    xt_ps = abps.tile([din, P], f32, tag="xtps")
                    nc.tensor.transpose(out=xt_ps[:], in_=xc[:], identity=ident[:])
                    xt = abp.tile([din, P], f32, tag="xt")
                    nc.scalar.activation(out=xt[:], in_=xt_ps[:], func=AF.Copy)
                    mml = abps.tile([P, D], f32, tag="mml")
                    nc.tensor.matmul(out=mml[:], lhsT=xt[:], rhs=wl[:], start=True, stop=True)
                    mmr = abps.tile([P, P], f32, tag="mmr")
                    nc.tensor.matmul(out=mmr[:], lhsT=xt[:], rhs=wr[:], start=True, stop=True)
                    sxl = abp.tile([P, D], f32, tag="sxl")
                    nc.vector.tensor_copy(out=sxl[:], in_=mml[:])
                    nc.sync.dma_start(out=xl_sh[t * P:(t + 1) * P, :], in_=sxl[:])
                    sxr = abp.tile([P, P], f32, tag="sxr")
                    nc.scalar.activation(out=sxr[:], in_=mmr[:], func=AF.Copy)
                    nc.sync.dma_start(out=xr_tab[t * P:(t + 1) * P, :], in_=sxr[:])

                # ---- gather table for xl across all cores
                if use_collective:
                    nc.gpsimd.collective_compute(
                        "AllGather", OP.bypass,
                        replica_groups=[list(range(ncores))],
                        ins=[xl_sh[:]], outs=[xl_full[:]],
                    )
                else:
                    nc.sync.dma_start(out=xl_full[:], in_=xl_sh[:])

                # ---- edge phase
                for blk in range(nblk):
                    ps = blkps.tile([P, W], f32, tag="ps")
                    # dma_gather per superchunk (512 idxs; SWDGE ring holds 1024 descs)
                    i16t = ep.tile([P, CH * P // 16], i16, tag="i16t")
                    nc.sync.dma_start(out=i16t[:], in_=dst16[blk, :, :])
                    xr_b = ep.tile([P, CH * P], f32, tag="xrb")
                    xr_v = xr_b[:].rearrange("p (c e) -> p c e", c=CH)
                    nsc_idx = SC * P // 16   # idx columns per superchunk
                    for sc in range(NSC):
                        nc.gpsimd.dma_gather(
                            out_ap=xr_v[:, sc * SC:(sc + 1) * SC, :],
                            in_ap=xr_tab[:],
                            idxs_ap=i16t[:, sc * nsc_idx:(sc + 1) * nsc_idx],
                            num_idxs=SC * P, num_idxs_reg=SC * P, elem_size=P)
                    for sc in range(NSC):
                        idst = ep.tile([P, 2 * SC], i32, tag="idst")
                        nc.sync.dma_start(out=idst[:], in_=ids[blk, sc, :, :])
                        xl_s = ep.tile([P, SC * D], f32, tag="xls")
                        for k in range(SC):
                            # HW indirect DMA: one gathered row per partition per call
                            nc.gpsimd.indirect_dma_start(
                                out=xl_s[:, k * D:(k + 1) * D], out_offset=None, in_=xl_full[:],
                                in_offset=bass.IndirectOffsetOnAxis(ap=idst[:, k:k + 1], axis=0))
                        g = ep.tile([P, SC * D], f32, tag="g")
                        nc.vector.tensor_tensor(
                            out=g[:].rearrange("p (s d) -> p s d", s=SC),
                            in0=xl_s[:].rearrange("p (s d) -> p s d", s=SC),
                            in1=xr_v[:, sc * SC:(sc + 1) * SC, 0:D],
                            op=OP.add)
                        gl = ep.tile([P, SC * D], f32, tag="gl")
                        # leaky_relu(g) = max(0.2*g, g) in one fused DVE op
                        nc.vector.scalar_tensor_tensor(
                            out=gl[:], in0=g[:], scalar=NEG_SLOPE, in1=g[:],
                            op0=OP.mult, op1=OP.max)
                        ge = ep.tile([P, SC * D], f32, tag="ge")
                        nc.vector.tensor_tensor(
                            out=ge[:].rearrange("p (s d) -> p s d", s=SC),
                            in0=gl[:].rearrange("p (s d) -> p s d", s=SC),
                            in1=attb[:].unsqueeze(1).to_broadcast([P, SC, D]),
                            op=OP.mult)
                        e = ep.tile([P, SC * H], f32, tag="e")
                        nc.vector.tensor_reduce(
                            out=e[:].rearrange("p (s h) -> p s h", s=SC),
                            in_=ge[:].rearrange("p (s h c) -> p s h c", s=SC, h=H),
                            axis=mybir.AxisListType.X, op=OP.add)
                        vals = ep.tile([P, SC * W], f32, tag="vals")
                        vals_v = vals[:].rearrange("p (s w) -> p s w", s=SC)
                        nc.scalar.activation(
                            out=vals_v[:, :, D:W],
                            in_=e[:].rearrange("p (s h) -> p s h", s=SC),
                            func=AF.Exp)
                        nc.vector.tensor_tensor(
                            out=vals_v[:, :, 0:D].rearrange("p s (h c) -> p s h c", h=H),
                            in0=xl_s[:].rearrange("p (s h c) -> p s h c", s=SC, h=H),
                            in1=vals_v[:, :, D:W].unsqueeze(3).to_broadcast([P, SC, H, C]),
                            op=OP.mult)
                        ot = ep.tile([P, SC * P], f32, tag="ot")
                        nc.vector.tensor_tensor(
                            out=ot[:].rearrange("p (s q) -> p s q", s=SC),
                            in0=idst[:, SC:2 * SC].bitcast(f32).unsqueeze(2).to_broadcast([P, SC, P]),
                            in1=iota[:].unsqueeze(1).to_broadcast([P, SC, P]),
                            op=OP.is_equal)
                        for k in range(SC):
                            ch = sc * SC + k
                            nc.tensor.matmul(
                                out=ps[:], lhsT=ot[:, k * P:(k + 1) * P],
                                rhs=vals[:, k * W:(k + 1) * W],
                                start=(ch == 0), stop=(ch == CH - 1))
                    # ---- finalize block: div by denom, +bias, elu, store
                    den = fp.tile([P, H], f32, tag="den")
                    nc.vector.tensor_scalar(
                        out=den[:], in0=ps[:, D:W], scalar1=1e-30, scalar2=None, op0=OP.max)
                    r = fp.tile([P, H], f32, tag="r")
                    nc.vector.reciprocal(out=r[:], in_=den[:])
                    o = fp.tile([P, D], f32, tag="o")
                    nc.vector.tensor_tensor(
                        out=o[:].rearrange("p (h c) -> p h c", h=H),
                        in0=ps[:, 0:D].rearrange("p (h c) -> p h c", h=H),
                        in1=r[:].unsqueeze(2).to_broadcast([P, H, C]),
                        op=OP.mult)
                    nc.vector.tensor_tensor(out=o[:], in0=o[:], in1=biasb[:], op=OP.add)
                    t1 = fp.tile([P, D], f32, tag="t1")
                    nc.vector.tensor_scalar(
                        out=t1[:], in0=o[:], scalar1=0.0, scalar2=None, op0=OP.min)
                    nc.scalar.activation(out=t1[:], in_=t1[:], func=AF.Exp)
                    nc.vector.tensor_scalar(
                        out=t1[:], in0=t1[:], scalar1=-1.0, scalar2=None, op0=OP.add)
                    nc.vector.tensor_tensor(out=o[:], in0=o[:], in1=t1[:], op=OP.max)
                    nc.sync.dma_start(out=x_out[blk * P:(blk + 1) * P, :], in_=o[:])
    nc.compile()
    return nc


# ---------------------------------------------------------------- entry

def make_inmaps(inputs, ncores):
    x = np.asarray(inputs['x'], np.float32)
    ei = np.asarray(inputs['edge_index'], np.int32)
    N, F = x.shape
    H, C = np.asarray(inputs['att0']).shape
    D = H * C
    L = 3
    loops = np.arange(N, dtype=np.int32)
    src = np.concatenate([ei[0], loops])
    dst = np.concatenate([ei[1], loops])
    Nshard, nblk, NSC, ids, dst16 = prep_edges(src, dst, N, ncores)
    xp = np.zeros((Nshard * ncores, F), np.float32)
    xp[:N] = x
    iota = np.broadcast_to(np.arange(P, dtype=np.float32), (P, P)).copy()
    ident = np.eye(P, dtype=np.float32)
    dims_in = [F] + [D] * (L - 1)
    base = {"ident": ident, "iota": iota}
    for l in range(L):
        base[f"Wl{l}"] = np.ascontiguousarray(np.asarray(inputs[f'Wl{l}'], np.float32))
        wr = np.asarray(inputs[f'Wr{l}'], np.float32)
        base[f"Wr{l}"] = np.concatenate([wr, np.zeros((wr.shape[0], P - D), np.float32)], 1)
        att = np.asarray(inputs[f'att{l}'], np.float32).reshape(1, D)
        base[f"attb{l}"] = np.broadcast_to(att, (P, D)).copy()
        b = np.asarray(inputs[f'b{l}'], np.float32).reshape(1, D)
        base[f"biasb{l}"] = np.broadcast_to(b, (P, D)).copy()
    in_maps = []
    for c in range(ncores):
        m = dict(base)
        m["x0"] = np.ascontiguousarray(xp[c * Nshard:(c + 1) * Nshard])
        m["ids"] = np.ascontiguousarray(ids[c])
        m["dst16"] = np.ascontiguousarray(dst16[c])
        in_maps.append(m)
    return in_maps, Nshard, nblk, NSC, dims_in, H, C, N, D


def kernel(**inputs):
    from concourse.bass_utils import run_bass_kernel_spmd
    ncores = 8
    in_maps, Nshard, nblk, NSC, dims_in, H, C, N, D = make_inmaps(inputs, ncores)
    nc = build_program(ncores, Nshard, nblk, NSC, dims_in, H, C, use_collective=True)
    res = run_bass_kernel_spmd(nc, in_maps, list(range(ncores)))
    out = np.concatenate([res.results[c]["y"] for c in range(ncores)], axis=0)
    return out[:N].astype(np.float32)


if __name__ == "__main__":
    pass



# revision 9
# speedup vs baseline: 1.7914x; 1.7914x over previous
"""GATv2 (3 layers, self-loops, segment softmax) on 8 Trainium2 NeuronCores.

v2 strategy: nodes sharded contiguously across 8 cores; edges routed to the
core owning their dst, sorted by dst, packed into 128-dst blocks x 128-edge
chunks. Per layer: node transform [xl|xr] = x @ [Wl|Wr] in bf16 on PE
(activations stored transposed so no PE transpose is needed), AllGather of the
bf16 xl table, then per block: ONE batched indirect DMA gathers all chunks'
xl[src] rows, per-edge xr[dst] comes from a one-hot matmul against the
block-local xr tile (dst rows of a block are a contiguous slice - no DMA
gather at all), scores on DVE/ACT, and a one-hot matmul does the per-block
segment reduction of softmax numerator+denominator in PSUM.

Self-contained: hardcodes problem shapes; no sibling imports.
"""
import numpy as np
import ml_dtypes

P = 128          # partitions / block size / chunk size
SC = 6           # chunks per superchunk (batched DVE elementwise)
GCALLS = 12      # indirect-DMA calls per block for the xl gather
NEG_SLOPE = 0.2
BF16 = ml_dtypes.bfloat16


# ---------------------------------------------------------------- host prep

def prep_edges(src, dst, N, ncores):
    """Route edges to dst-owning cores, sort by dst, pack into block/chunk slots.

    Returns (Nshard, nblk, NSC, ids, oh2):
      ids int32 [ncores, nblk, P, 2*CH]: cols [0:CH] global src id,
        [CH:2*CH] float32-bitcast block-relative dst (1e6 sentinel pad).
      oh2 bf16 [ncores, nblk, P, CH*P]: transposed one-hot
        oh2[c,b,q,ch*P+e] = (dst_rel[slot ch,e] == q); sentinel cols are 0.
    """
    Nshard = ((N + ncores * P - 1) // (ncores * P)) * P
    nblk = Nshard // P
    core = dst // Nshard
    percore = []
    maxch = 1
    for c in range(ncores):
        m = core == c
        s = src[m]
        dl = (dst[m] - c * Nshard).astype(np.int64)
        o = np.argsort(dl, kind='stable')
        s, dl = s[o], dl[o]
        blk = dl // P
        counts = np.bincount(blk, minlength=nblk)
        maxch = max(maxch, int(np.max((counts + P - 1) // P)))
        percore.append((s, dl, blk, counts))
    NSC = (maxch + SC - 1) // SC
    CH = NSC * SC
    ids = np.zeros((ncores, nblk, CH, P, 2), np.int32)
    sentinel = np.float32(1e6).view(np.int32)
    ids[:, :, :, :, 1] = sentinel
    oh2 = np.zeros((ncores, nblk, P, CH * P), BF16)
    for c in range(ncores):
        s, dl, blk, counts = percore[c]
        starts = np.zeros(nblk, np.int64)
        starts[1:] = np.cumsum(counts)[:-1]
        pos = np.arange(len(s)) - starts[blk]        # rank within block
        ch = pos // P
        p = pos % P
        ids[c, blk, ch, p, 0] = s
        rel = (dl - blk * P).astype(np.int64)
        ids[c, blk, ch, p, 1] = rel.astype(np.float32).view(np.int32)
        oh2[c, blk, rel, ch * P + p] = 1
    # ids: [c, nblk, CH, P, 2] -> [c, nblk, P, 2*CH] with src cols then rel cols
    ids = ids.transpose(0, 1, 3, 4, 2)               # [c, nblk, P, 2, CH]
    ids = ids.reshape(ncores, nblk, P, 2 * CH)
    return Nshard, nblk, NSC, np.ascontiguousarray(ids), oh2


# ---------------------------------------------------------------- bass build

def build_program(ncores, Nshard, nblk, NSC, dims_in, H, C, use_collective=True):
    import concourse.bass as bass
    import concourse.mybir as mybir
    from concourse import bacc
    from concourse.tile import TileContext

    D = H * C
    W = D + H
    L = len(dims_in)
    CH = NSC * SC
    Np = Nshard * ncores
    f32, i32, bf16 = mybir.dt.float32, mybir.dt.int32, mybir.dt.bfloat16
    AF = mybir.ActivationFunctionType
    OP = mybir.AluOpType

    nc = bacc.Bacc()
    x0T = nc.declare_dram_parameter("x0T", [dims_in[0], Nshard], bf16, isOutput=False)
    ids = nc.declare_dram_parameter("ids", [nblk, P, 2 * CH], i32, isOutput=False)
    oh2 = nc.declare_dram_parameter("oh2", [nblk, P, CH * P], bf16, isOutput=False)
    wparams = []
    for l in range(L):
        din = dims_in[l]
        wparams.append((
            nc.declare_dram_parameter(f"Wlr{l}", [din, 2 * D], bf16, isOutput=False),
            nc.declare_dram_parameter(f"attb{l}", [P, SC * D], bf16, isOutput=False),
            nc.declare_dram_parameter(f"biasb{l}", [P, D], f32, isOutput=False),
        ))
    ident_in = nc.declare_dram_parameter("ident", [P, P], bf16, isOutput=False)
    iota_in = nc.declare_dram_parameter("iota", [P, P], f32, isOutput=False)
    y = nc.declare_dram_parameter("y", [Nshard, D], f32, isOutput=True)

    xl_sh = nc.dram_tensor("xl_sh", [Nshard, D], bf16)
    xl_full = nc.dram_tensor("xl_full", [Np, D], bf16, addr_space="Shared")
    xmT = [nc.dram_tensor(f"xmT{i}", [D, Nshard], bf16) for i in range(L - 1)]

    with TileContext(nc) as tc:
        with (
            tc.tile_pool(name="const", bufs=1) as cp,
            tc.tile_pool(name="xr_res", bufs=1) as xrp,
            tc.tile_pool(name="node", bufs=3) as np_,
            tc.tile_pool(name="nodeps", bufs=2, space="PSUM") as nps_,
            tc.tile_pool(name="edge", bufs=3) as ep,
            tc.tile_pool(name="xrps", bufs=2, space="PSUM") as xps,
            tc.tile_pool(name="blkps", bufs=2, space="PSUM") as blkps,
            tc.tile_pool(name="fin", bufs=2) as fp,
        ):
            ident = cp.tile([P, P], bf16, tag="ident")
            nc.sync.dma_start(out=ident[:], in_=ident_in[:])
            iota = cp.tile([P, P], f32, tag="iota")
            nc.sync.dma_start(out=iota[:], in_=iota_in[:])
            wts = []
            for l in range(L):
                din = dims_in[l]
                wlr = cp.tile([din, 2 * D], bf16, tag=f"wlr{l}")
                nc.sync.dma_start(out=wlr[:], in_=wparams[l][0][:])
                attb = cp.tile([P, SC * D], bf16, tag=f"attb{l}")
                nc.sync.dma_start(out=attb[:], in_=wparams[l][1][:])
                biasb = cp.tile([P, D], f32, tag=f"biasb{l}")
                nc.sync.dma_start(out=biasb[:], in_=wparams[l][2][:])
                wts.append((wlr, attb, biasb))

            for l in range(L):
                din = dims_in[l]
                x_out = y if l == L - 1 else xmT[l]
                wlr, attb, biasb = wts[l]

                # ---- node transform: [xl|xr] = x @ [Wl|Wr]  (bf16, PE)
                xlr_all = []
                for t in range(nblk):
                    xt = np_.tile([din, P], bf16, tag="xt")
                    if l == 0:
                        nc.sync.dma_start(out=xt[:], in_=x0T[:, t * P:(t + 1) * P])
                    else:
                        nc.sync.dma_start(out=xt[:], in_=xmT[l - 1][:, t * P:(t + 1) * P])
                    mm = nps_.tile([P, 2 * D], f32, tag="mm")
                    nc.tensor.matmul(out=mm[:], lhsT=xt[:], rhs=wlr[:], start=True, stop=True)
                    xlr = xrp.tile([P, 2 * D], bf16, tag=f"xlr{t}")
                    nc.scalar.activation(out=xlr[:], in_=mm[:], func=AF.Copy)
                    nc.sync.dma_start(out=xl_sh[t * P:(t + 1) * P, :], in_=xlr[:, 0:D])
                    xlr_all.append(xlr)

                # ---- gather table for xl across all cores
                if use_collective:
                    nc.gpsimd.collective_compute(
                        "AllGather", OP.bypass,
                        replica_groups=[list(range(ncores))],
                        ins=[xl_sh[:]], outs=[xl_full[:]],
                    )
                else:
                    nc.sync.dma_start(out=xl_full[:], in_=xl_sh[:])

                # ---- edge phase
                for blk in range(nblk):
                    idst = ep.tile([P, 2 * CH], i32, tag="idst")
                    nc.sync.dma_start(out=idst[:], in_=ids[blk, :, :])
                    oh2t = ep.tile([P, CH * P], bf16, tag="oh2t")
                    nc.sync.dma_start(out=oh2t[:], in_=oh2[blk, :, :])
                    xl_s = ep.tile([P, CH * D], bf16, tag="xls")
                    gg = CH // GCALLS
                    for gci in range(GCALLS):
                        nc.gpsimd.indirect_dma_start(
                            out=xl_s[:, gci * gg * D:(gci + 1) * gg * D],
                            out_offset=None, in_=xl_full[:],
                            in_offset=bass.IndirectOffsetOnAxis(
                                ap=idst[:, gci * gg:(gci + 1) * gg], axis=0))
                    xrb = xlr_all[blk]
                    ps = blkps.tile([P, W], f32, tag="ps")
                    for sc in range(NSC):
                        # xr[dst] for SC chunks via one-hot matmul (PE, no DMA)
                        xr_ps = xps.tile([P, SC * D], f32, tag="xrps")
                        for k in range(SC):
                            ch = sc * SC + k
                            nc.tensor.matmul(
                                out=xr_ps[:, k * D:(k + 1) * D],
                                lhsT=oh2t[:, ch * P:(ch + 1) * P],
                                rhs=xrb[:, D:2 * D], start=True, stop=True)
                        xsl = xl_s[:, sc * SC * D:(sc + 1) * SC * D]
                        xr_sb = ep.tile([P, SC * D], bf16, tag="xrsb")
                        nc.scalar.activation(out=xr_sb[:], in_=xr_ps[:], func=AF.Copy)
                        g = ep.tile([P, SC * D], bf16, tag="g")
                        nc.vector.tensor_tensor(out=g[:], in0=xsl, in1=xr_sb[:], op=OP.add)
                        gl = ep.tile([P, SC * D], bf16, tag="gl")
                        # leaky_relu(g) = max(0.2*g, g) in one fused DVE op
                        nc.vector.scalar_tensor_tensor(
                            out=gl[:], in0=g[:], scalar=NEG_SLOPE, in1=g[:],
                            op0=OP.mult, op1=OP.max)
                        ge = ep.tile([P, SC * D], bf16, tag="ge")
                        nc.vector.tensor_tensor(out=ge[:], in0=gl[:], in1=attb[:], op=OP.mult)
                        e = ep.tile([P, SC * H], f32, tag="e")
                        nc.vector.tensor_reduce(
                            out=e[:].rearrange("p (s h) -> p s h", s=SC),
                            in_=ge[:].rearrange("p (s h c) -> p s h c", s=SC, h=H),
                            axis=mybir.AxisListType.X, op=OP.add)
                        vals = ep.tile([P, SC * W], bf16, tag="vals")
                        vals_v = vals[:].rearrange("p (s w) -> p s w", s=SC)
                        nc.scalar.activation(
                            out=vals_v[:, :, D:W],
                            in_=e[:].rearrange("p (s h) -> p s h", s=SC),
                            func=AF.Exp)
                        nc.vector.tensor_tensor(
                            out=vals_v[:, :, 0:D].rearrange("p s (h c) -> p s h c", h=H),
                            in0=xsl.rearrange("p (s h c) -> p s h c", s=SC, h=H),
                            in1=vals_v[:, :, D:W].unsqueeze(3).to_broadcast([P, SC, H, C]),
                            op=OP.mult)
                        ot = ep.tile([P, SC * P], bf16, tag="ot")
                        nc.vector.tensor_tensor(
                            out=ot[:].rearrange("p (s q) -> p s q", s=SC),
                            in0=idst[:, CH + sc * SC:CH + (sc + 1) * SC].bitcast(f32)
                                .unsqueeze(2).to_broadcast([P, SC, P]),
                            in1=iota[:].unsqueeze(1).to_broadcast([P, SC, P]),
                            op=OP.is_equal)
                        for k in range(SC):
                            ch = sc * SC + k
                            nc.tensor.matmul(
                                out=ps[:], lhsT=ot[:, k * P:(k + 1) * P],
                                rhs=vals[:, k * W:(k + 1) * W],
                                start=(ch == 0), stop=(ch == CH - 1))
                    # ---- finalize block: div by denom, +bias, elu, store
                    den = fp.tile([P, H], f32, tag="den")
                    nc.vector.tensor_scalar(
                        out=den[:], in0=ps[:, D:W], scalar1=1e-30, scalar2=None, op0=OP.max)
                    r = fp.tile([P, H], f32, tag="r")
                    nc.vector.reciprocal(out=r[:], in_=den[:])
                    o = fp.tile([P, D], f32, tag="o")
                    nc.vector.tensor_tensor(
                        out=o[:].rearrange("p (h c) -> p h c", h=H),
                        in0=ps[:, 0:D].rearrange("p (h c) -> p h c", h=H),
                        in1=r[:].unsqueeze(2).to_broadcast([P, H, C]),
                        op=OP.mult)
                    nc.vector.tensor_tensor(out=o[:], in0=o[:], in1=biasb[:], op=OP.add)
                    t1 = fp.tile([P, D], f32, tag="t1")
                    nc.vector.tensor_scalar(
                        out=t1[:], in0=o[:], scalar1=0.0, scalar2=None, op0=OP.min)
                    nc.scalar.activation(out=t1[:], in_=t1[:], func=AF.Exp)
                    nc.vector.tensor_scalar(
                        out=t1[:], in0=t1[:], scalar1=-1.0, scalar2=None, op0=OP.add)
                    nc.vector.tensor_tensor(out=o[:], in0=o[:], in1=t1[:], op=OP.max)
                    if l == L - 1:
                        nc.sync.dma_start(out=y[blk * P:(blk + 1) * P, :], in_=o[:])
                    else:
                        ob = fp.tile([P, D], bf16, tag="ob")
                        nc.vector.tensor_copy(out=ob[:], in_=o[:])
                        obT_ps = nps_.tile([D, P], bf16, tag="obT")
                        nc.tensor.transpose(out=obT_ps[:], in_=ob[:], identity=ident[:])
                        obT = fp.tile([D, P], bf16, tag="obTs")
                        nc.scalar.activation(out=obT[:], in_=obT_ps[:], func=AF.Copy)
                        nc.sync.dma_start(out=x_out[:, blk * P:(blk + 1) * P], in_=obT[:])
    nc.compile()
    return nc


# ---------------------------------------------------------------- entry

def make_inmaps(inputs, ncores):
    x = np.asarray(inputs['x'], np.float32)
    ei = np.asarray(inputs['edge_index'], np.int32)
    N, F = x.shape
    H, C = np.asarray(inputs['att0']).shape
    D = H * C
    L = 3
    loops = np.arange(N, dtype=np.int32)
    src = np.concatenate([ei[0], loops])
    dst = np.concatenate([ei[1], loops])
    Nshard, nblk, NSC, ids, oh2 = prep_edges(src, dst, N, ncores)
    xp = np.zeros((Nshard * ncores, F), np.float32)
    xp[:N] = x
    iota = np.broadcast_to(np.arange(P, dtype=np.float32), (P, P)).copy()
    ident = np.eye(P, dtype=BF16)
    dims_in = [F] + [D] * (L - 1)
    base = {"ident": ident, "iota": iota}
    for l in range(L):
        wl = np.asarray(inputs[f'Wl{l}'], np.float32)
        wr = np.asarray(inputs[f'Wr{l}'], np.float32)
        base[f"Wlr{l}"] = np.concatenate([wl, wr], axis=1).astype(BF16)
        att = np.asarray(inputs[f'att{l}'], np.float32).reshape(1, D)
        base[f"attb{l}"] = np.broadcast_to(np.tile(att, (1, SC)), (P, SC * D)).astype(BF16)
        b = np.asarray(inputs[f'b{l}'], np.float32).reshape(1, D)
        base[f"biasb{l}"] = np.broadcast_to(b, (P, D)).copy()
    in_maps = []
    for c in range(ncores):
        m = dict(base)
        m["x0T"] = np.ascontiguousarray(
            xp[c * Nshard:(c + 1) * Nshard].T).astype(BF16)
        m["ids"] = np.ascontiguousarray(ids[c])
        m["oh2"] = np.ascontiguousarray(oh2[c])
        in_maps.append(m)
    return in_maps, Nshard, nblk, NSC, dims_in, H, C, N, D


def kernel(**inputs):
    from concourse.bass_utils import run_bass_kernel_spmd
    ncores = 8
    in_maps, Nshard, nblk, NSC, dims_in, H, C, N, D = make_inmaps(inputs, ncores)
    nc = build_program(ncores, Nshard, nblk, NSC, dims_in, H, C, use_collective=True)
    res = run_bass_kernel_spmd(nc, in_maps, list(range(ncores)))
    out = np.concatenate([res.results[c]["y"] for c in range(ncores)], axis=0)
    return out[:N].astype(np.float32)


if __name__ == "__main__":
    pass


# revision 14
# speedup vs baseline: 1.9592x; 1.0937x over previous
"""GATv2 (3 layers, self-loops, segment softmax) on 8 Trainium2 NeuronCores.

v3 strategy: nodes sharded contiguously across 8 cores; non-self-loop edges
routed to the core owning their dst, sorted by dst, packed into 128-dst
blocks x 128-edge chunks (variable chunk count per block, SPMD-max across
cores). Per layer: node transform [xl|xr] = x @ [Wl|Wr] in bf16 on PE
(activations stored transposed so no PE transpose is needed), AllGather of
the bf16 xl table, then per block: one [P,1]-offset indirect DMA per chunk
gathers xl[src] rows (the HW SWDGE lowering only honors one offset per
partition), per-edge xr[dst] comes from a one-hot matmul against the
block-local xr tile, scores on DVE/ACT, and a one-hot matmul does the
per-block segment reduction of softmax numerator+denominator in PSUM.
Self-loops never enter the edge stream: the self contribution for dst node
p of block b is computed in the finalize directly from the block's local
xl/xr rows (partition p) and added to the PSUM num/denom.

Self-contained: hardcodes problem shapes; no sibling imports.
"""
import numpy as np
import ml_dtypes

P = 128          # partitions / block size / chunk size
SC = 6           # chunks per superchunk (batched DVE elementwise)
NEG_SLOPE = 0.2
BF16 = ml_dtypes.bfloat16


# ---------------------------------------------------------------- host prep

def prep_edges(src, dst, N, ncores):
    """Route edges to dst-owning cores, sort by dst, pack into block/chunk slots.

    Returns (Nshard, nblk, chks, ids, oh2):
      chks int list [nblk]: chunks used per block (max across cores).
      ids int32 [ncores, nblk, P, 2*CHmax]: cols [0:CHmax] global src id,
        [CHmax:2*CHmax] float32-bitcast block-relative dst (1e6 sentinel pad).
      oh2 bf16 [ncores, nblk, P, CHmax*P]: transposed one-hot
        oh2[c,b,q,ch*P+e] = (dst_rel[slot ch,e] == q); sentinel cols are 0.
    """
    Nshard = ((N + ncores * P - 1) // (ncores * P)) * P
    nblk = Nshard // P
    core = dst // Nshard
    percore = []
    cnts = np.zeros((ncores, nblk), np.int64)
    for c in range(ncores):
        m = core == c
        s = src[m]
        dl = (dst[m] - c * Nshard).astype(np.int64)
        o = np.argsort(dl, kind='stable')
        s, dl = s[o], dl[o]
        blk = dl // P
        counts = np.bincount(blk, minlength=nblk)
        cnts[c] = counts
        percore.append((s, dl, blk, counts))
    chks = np.maximum(1, (cnts.max(axis=0) + P - 1) // P)   # [nblk], SPMD max
    CH = int(chks.max())
    ids = np.zeros((ncores, nblk, CH, P, 2), np.int32)
    sentinel = np.float32(1e6).view(np.int32)
    ids[:, :, :, :, 1] = sentinel
    oh2 = np.zeros((ncores, nblk, P, CH * P), BF16)
    for c in range(ncores):
        s, dl, blk, counts = percore[c]
        starts = np.zeros(nblk, np.int64)
        starts[1:] = np.cumsum(counts)[:-1]
        pos = np.arange(len(s)) - starts[blk]        # rank within block
        ch = pos // P
        p = pos % P
        ids[c, blk, ch, p, 0] = s
        rel = (dl - blk * P).astype(np.int64)
        ids[c, blk, ch, p, 1] = rel.astype(np.float32).view(np.int32)
        oh2[c, blk, rel, ch * P + p] = 1
    # ids: [c, nblk, CH, P, 2] -> [c, nblk, P, 2*CH] with src cols then rel cols
    ids = ids.transpose(0, 1, 3, 4, 2)               # [c, nblk, P, 2, CH]
    ids = ids.reshape(ncores, nblk, P, 2 * CH)
    return Nshard, nblk, [int(v) for v in chks], np.ascontiguousarray(ids), oh2


# ---------------------------------------------------------------- bass build

def build_program(ncores, Nshard, nblk, chks, dims_in, H, C, use_collective=True):
    import concourse.bass as bass
    import concourse.mybir as mybir
    from concourse import bacc
    from concourse.tile import TileContext

    D = H * C
    W = D + H
    L = len(dims_in)
    CH = max(chks)
    Np = Nshard * ncores
    f32, i32, bf16 = mybir.dt.float32, mybir.dt.int32, mybir.dt.bfloat16
    AF = mybir.ActivationFunctionType
    OP = mybir.AluOpType

    nc = bacc.Bacc()
    x0T = nc.declare_dram_parameter("x0T", [dims_in[0], Nshard], bf16, isOutput=False)
    ids = nc.declare_dram_parameter("ids", [nblk, P, 2 * CH], i32, isOutput=False)
    oh2 = nc.declare_dram_parameter("oh2", [nblk, P, CH * P], bf16, isOutput=False)
    wparams = []
    for l in range(L):
        din = dims_in[l]
        wparams.append((
            nc.declare_dram_parameter(f"Wlr{l}", [din, 2 * D], bf16, isOutput=False),
            nc.declare_dram_parameter(f"attb{l}", [P, SC * D], bf16, isOutput=False),
            nc.declare_dram_parameter(f"biasb{l}", [P, D], f32, isOutput=False),
        ))
    ident_in = nc.declare_dram_parameter("ident", [P, P], bf16, isOutput=False)
    iota_in = nc.declare_dram_parameter("iota", [P, P], f32, isOutput=False)
    y = nc.declare_dram_parameter("y", [Nshard, D], f32, isOutput=True)

    xl_sh = nc.dram_tensor("xl_sh", [Nshard, D], bf16)
    xl_full = nc.dram_tensor("xl_full", [Np, D], bf16, addr_space="Shared")
    xmT = [nc.dram_tensor(f"xmT{i}", [D, Nshard], bf16) for i in range(L - 1)]

    with TileContext(nc) as tc:
        with (
            tc.tile_pool(name="const", bufs=1) as cp,
            tc.tile_pool(name="xr_res", bufs=1) as xrp,
            tc.tile_pool(name="node", bufs=3) as np_,
            tc.tile_pool(name="nodeps", bufs=2, space="PSUM") as nps_,
            tc.tile_pool(name="edge", bufs=3) as ep,
            tc.tile_pool(name="xrps", bufs=2, space="PSUM") as xps,
            tc.tile_pool(name="blkps", bufs=2, space="PSUM") as blkps,
            tc.tile_pool(name="fin", bufs=2) as fp,
        ):
            ident = cp.tile([P, P], bf16, tag="ident")
            nc.sync.dma_start(out=ident[:], in_=ident_in[:])
            iota = cp.tile([P, P], f32, tag="iota")
            nc.sync.dma_start(out=iota[:], in_=iota_in[:])
            wts = []
            for l in range(L):
                din = dims_in[l]
                wlr = cp.tile([din, 2 * D], bf16, tag=f"wlr{l}")
                nc.sync.dma_start(out=wlr[:], in_=wparams[l][0][:])
                attb = cp.tile([P, SC * D], bf16, tag=f"attb{l}")
                nc.sync.dma_start(out=attb[:], in_=wparams[l][1][:])
                biasb = cp.tile([P, D], f32, tag=f"biasb{l}")
                nc.sync.dma_start(out=biasb[:], in_=wparams[l][2][:])
                wts.append((wlr, attb, biasb))

            for l in range(L):
                din = dims_in[l]
                x_out = y if l == L - 1 else xmT[l]
                wlr, attb, biasb = wts[l]

                # ---- node transform: [xl|xr] = x @ [Wl|Wr]  (bf16, PE)
                xlr_all = []
                for t in range(nblk):
                    xt = np_.tile([din, P], bf16, tag="xt")
                    if l == 0:
                        nc.sync.dma_start(out=xt[:], in_=x0T[:, t * P:(t + 1) * P])
                    else:
                        nc.sync.dma_start(out=xt[:], in_=xmT[l - 1][:, t * P:(t + 1) * P])
                    mm = nps_.tile([P, 2 * D], f32, tag="mm")
                    nc.tensor.matmul(out=mm[:], lhsT=xt[:], rhs=wlr[:], start=True, stop=True)
                    xlr = xrp.tile([P, 2 * D], bf16, tag=f"xlr{t}")
                    nc.scalar.activation(out=xlr[:], in_=mm[:], func=AF.Copy)
                    nc.sync.dma_start(out=xl_sh[t * P:(t + 1) * P, :], in_=xlr[:, 0:D])
                    xlr_all.append(xlr)

                # ---- gather table for xl across all cores
                if use_collective:
                    nc.gpsimd.collective_compute(
                        "AllGather", OP.bypass,
                        replica_groups=[list(range(ncores))],
                        ins=[xl_sh[:]], outs=[xl_full[:]],
                    )
                else:
                    nc.sync.dma_start(out=xl_full[:], in_=xl_sh[:])

                # ---- edge phase
                for blk in range(nblk):
                    chk = chks[blk]
                    idst = ep.tile([P, 2 * CH], i32, tag="idst")
                    nc.sync.dma_start(out=idst[:, 0:chk], in_=ids[blk, :, 0:chk])
                    nc.sync.dma_start(out=idst[:, CH:CH + chk], in_=ids[blk, :, CH:CH + chk])
                    oh2t = ep.tile([P, CH * P], bf16, tag="oh2t")
                    nc.sync.dma_start(out=oh2t[:, 0:chk * P], in_=oh2[blk, :, 0:chk * P])
                    xl_s = ep.tile([P, CH * D], bf16, tag="xls")
                    for ch in range(chk):
                        nc.gpsimd.indirect_dma_start(
                            out=xl_s[:, ch * D:(ch + 1) * D],
                            out_offset=None, in_=xl_full[:],
                            in_offset=bass.IndirectOffsetOnAxis(
                                ap=idst[:, ch:ch + 1], axis=0))
                    xrb = xlr_all[blk]
                    ps = blkps.tile([P, W], f32, tag="ps")
                    nsc = (chk + SC - 1) // SC
                    for sc in range(nsc):
                        kk = min(SC, chk - sc * SC)
                        # xr[dst] for kk chunks via one-hot matmul (PE, no DMA)
                        xr_ps = xps.tile([P, SC * D], f32, tag="xrps")
                        for k in range(kk):
                            ch = sc * SC + k
                            nc.tensor.matmul(
                                out=xr_ps[:, k * D:(k + 1) * D],
                                lhsT=oh2t[:, ch * P:(ch + 1) * P],
                                rhs=xrb[:, D:2 * D], start=True, stop=True)
                        xsl = xl_s[:, sc * SC * D:(sc * SC + kk) * D]
                        xr_sb = ep.tile([P, SC * D], bf16, tag="xrsb")
                        nc.scalar.activation(out=xr_sb[:, 0:kk * D], in_=xr_ps[:, 0:kk * D],
                                             func=AF.Copy)
                        g = ep.tile([P, SC * D], bf16, tag="g")
                        nc.vector.tensor_tensor(out=g[:, 0:kk * D], in0=xsl,
                                                in1=xr_sb[:, 0:kk * D], op=OP.add)
                        gl = ep.tile([P, SC * D], bf16, tag="gl")
                        # leaky_relu(g) = max(0.2*g, g) in one fused DVE op
                        nc.vector.scalar_tensor_tensor(
                            out=gl[:, 0:kk * D], in0=g[:, 0:kk * D], scalar=NEG_SLOPE,
                            in1=g[:, 0:kk * D], op0=OP.mult, op1=OP.max)
                        ge = ep.tile([P, SC * D], bf16, tag="ge")
                        nc.vector.tensor_tensor(out=ge[:, 0:kk * D], in0=gl[:, 0:kk * D],
                                                in1=attb[:, 0:kk * D], op=OP.mult)
                        e = ep.tile([P, SC * H], f32, tag="e")
                        nc.vector.tensor_reduce(
                            out=e[:, 0:kk * H],
                            in_=ge[:, 0:kk * D].rearrange("p (sh c) -> p sh c", c=C),
                            axis=mybir.AxisListType.X, op=OP.add)
                        vals = ep.tile([P, SC * W], bf16, tag="vals")
                        vals_v = vals[:].rearrange("p (s w) -> p s w", s=SC)
                        nc.scalar.activation(
                            out=vals_v[:, 0:kk, D:W],
                            in_=e[:, 0:kk * H].rearrange("p (s h) -> p s h", s=kk),
                            func=AF.Exp)
                        nc.vector.tensor_tensor(
                            out=vals_v[:, 0:kk, 0:D].rearrange("p s (h c) -> p s h c", h=H),
                            in0=xsl.rearrange("p (s h c) -> p s h c", s=kk, h=H),
                            in1=vals_v[:, 0:kk, D:W].unsqueeze(3).to_broadcast([P, kk, H, C]),
                            op=OP.mult)
                        ot = ep.tile([P, SC * P], bf16, tag="ot")
                        nc.vector.tensor_tensor(
                            out=ot[:, 0:kk * P].rearrange("p (s q) -> p s q", s=kk),
                            in0=idst[:, CH + sc * SC:CH + sc * SC + kk].bitcast(f32)
                                .unsqueeze(2).to_broadcast([P, kk, P]),
                            in1=iota[:].unsqueeze(1).to_broadcast([P, kk, P]),
                            op=OP.is_equal)
                        for k in range(kk):
                            ch = sc * SC + k
                            nc.tensor.matmul(
                                out=ps[:], lhsT=ot[:, k * P:(k + 1) * P],
                                rhs=vals[:, k * W:(k + 1) * W],
                                start=(ch == 0), stop=(ch == chk - 1))
                    # ---- finalize block: self-loop term, div by denom, +bias, elu
                    gs = fp.tile([P, D], bf16, tag="gs")
                    nc.vector.tensor_tensor(out=gs[:], in0=xrb[:, 0:D], in1=xrb[:, D:2 * D],
                                            op=OP.add)
                    nc.vector.scalar_tensor_tensor(
                        out=gs[:], in0=gs[:], scalar=NEG_SLOPE, in1=gs[:],
                        op0=OP.mult, op1=OP.max)
                    nc.vector.tensor_tensor(out=gs[:], in0=gs[:], in1=attb[:, 0:D], op=OP.mult)
                    es = fp.tile([P, H], f32, tag="es")
                    nc.vector.tensor_reduce(
                        out=es[:], in_=gs[:].rearrange("p (h c) -> p h c", h=H),
                        axis=mybir.AxisListType.X, op=OP.add)
                    pse = fp.tile([P, H], f32, tag="pse")
                    nc.scalar.activation(out=pse[:], in_=es[:], func=AF.Exp)
                    den = fp.tile([P, H], f32, tag="den")
                    nc.vector.tensor_tensor(out=den[:], in0=ps[:, D:W], in1=pse[:], op=OP.add)
                    r = fp.tile([P, H], f32, tag="r")
                    nc.vector.reciprocal(out=r[:], in_=den[:])
                    num = fp.tile([P, D], f32, tag="num")
                    nc.vector.tensor_tensor(
                        out=num[:].rearrange("p (h c) -> p h c", h=H),
                        in0=xrb[:, 0:D].rearrange("p (h c) -> p h c", h=H),
                        in1=pse[:].unsqueeze(2).to_broadcast([P, H, C]),
                        op=OP.mult)
                    nc.vector.tensor_tensor(out=num[:], in0=num[:], in1=ps[:, 0:D], op=OP.add)
                    o = fp.tile([P, D], f32, tag="o")
                    nc.vector.tensor_tensor(
                        out=o[:].rearrange("p (h c) -> p h c", h=H),
                        in0=num[:].rearrange("p (h c) -> p h c", h=H),
                        in1=r[:].unsqueeze(2).to_broadcast([P, H, C]),
                        op=OP.mult)
                    nc.vector.tensor_tensor(out=o[:], in0=o[:], in1=biasb[:], op=OP.add)
                    t1 = fp.tile([P, D], f32, tag="t1")
                    nc.vector.tensor_scalar(
                        out=t1[:], in0=o[:], scalar1=0.0, scalar2=None, op0=OP.min)
                    nc.scalar.activation(out=t1[:], in_=t1[:], func=AF.Exp)
                    nc.vector.tensor_scalar(
                        out=t1[:], in0=t1[:], scalar1=-1.0, scalar2=None, op0=OP.add)
                    nc.vector.tensor_tensor(out=o[:], in0=o[:], in1=t1[:], op=OP.max)
                    if l == L - 1:
                        nc.sync.dma_start(out=y[blk * P:(blk + 1) * P, :], in_=o[:])
                    else:
                        ob = fp.tile([P, D], bf16, tag="ob")
                        nc.vector.tensor_copy(out=ob[:], in_=o[:])
                        obT_ps = nps_.tile([D, P], bf16, tag="obT")
                        nc.tensor.transpose(out=obT_ps[:], in_=ob[:], identity=ident[:])
                        obT = fp.tile([D, P], bf16, tag="obTs")
                        nc.scalar.activation(out=obT[:], in_=obT_ps[:], func=AF.Copy)
                        nc.sync.dma_start(out=x_out[:, blk * P:(blk + 1) * P], in_=obT[:])
    nc.compile()
    return nc


# ---------------------------------------------------------------- entry

def make_inmaps(inputs, ncores):
    x = np.asarray(inputs['x'], np.float32)
    ei = np.asarray(inputs['edge_index'], np.int32)
    N, F = x.shape
    H, C = np.asarray(inputs['att0']).shape
    D = H * C
    L = 3
    # the synthetic self-loops (PyG add_self_loops) are handled analytically in
    # the kernel finalize; natural (i,i) edges stay in the edge stream
    src = ei[0]
    dst = ei[1]
    Nshard, nblk, chks, ids, oh2 = prep_edges(src, dst, N, ncores)
    xp = np.zeros((Nshard * ncores, F), np.float32)
    xp[:N] = x
    iota = np.broadcast_to(np.arange(P, dtype=np.float32), (P, P)).copy()
    ident = np.eye(P, dtype=BF16)
    dims_in = [F] + [D] * (L - 1)
    base = {"ident": ident, "iota": iota}
    for l in range(L):
        wl = np.asarray(inputs[f'Wl{l}'], np.float32)
        wr = np.asarray(inputs[f'Wr{l}'], np.float32)
        base[f"Wlr{l}"] = np.concatenate([wl, wr], axis=1).astype(BF16)
        att = np.asarray(inputs[f'att{l}'], np.float32).reshape(1, D)
        base[f"attb{l}"] = np.broadcast_to(np.tile(att, (1, SC)), (P, SC * D)).astype(BF16)
        b = np.asarray(inputs[f'b{l}'], np.float32).reshape(1, D)
        base[f"biasb{l}"] = np.broadcast_to(b, (P, D)).copy()
    in_maps = []
    for c in range(ncores):
        mm = dict(base)
        mm["x0T"] = np.ascontiguousarray(
            xp[c * Nshard:(c + 1) * Nshard].T).astype(BF16)
        mm["ids"] = np.ascontiguousarray(ids[c])
        mm["oh2"] = np.ascontiguousarray(oh2[c])
        in_maps.append(mm)
    return in_maps, Nshard, nblk, chks, dims_in, H, C, N, D


def kernel(**inputs):
    from concourse.bass_utils import run_bass_kernel_spmd
    ncores = 8
    in_maps, Nshard, nblk, chks, dims_in, H, C, N, D = make_inmaps(inputs, ncores)
    nc = build_program(ncores, Nshard, nblk, chks, dims_in, H, C, use_collective=True)
    res = run_bass_kernel_spmd(nc, in_maps, list(range(ncores)))
    out = np.concatenate([res.results[c]["y"] for c in range(ncores)], axis=0)
    return out[:N].astype(np.float32)


if __name__ == "__main__":
    pass
